# revision 50
# baseline (speedup 1.0000x reference)
"""Trainium2 Bass kernel for a pre-norm transformer block (B=1, T=4096, C=1024, H=16).

Sharding (8 cores): head-tensor-parallel attention (2 heads/core) with
sequence-parallel LayerNorm statistics and sequence-local MLP.
Activations are kept transposed on-chip ([C, T] with C on partitions)
so every matmul contracts over the partition axis with K=128 chunks.

Schedule (v4):
  1. LN1 statistics on own xT columns -> tiny AllGather trigger ASAP.
  2. While the collective (and the cross-core launch-skew barrier) is in
     flight, accumulate the *unscaled* QKV for the full sequence
     (qkv_raw = W'^T x with W' = diag(ln1_w) @ w_qkv folded host-side)
     straight into the q/k/v tiles.  LN is affine in x, so the final
     qkv = rstd*(qkv_raw - mu*colsum(W')) is applied in place (DVE add
     + GPSIMD scale) once the AllGather'd (rstd, mu) land.
  3. Causal flash attention with the PE in 64-row tiling mode
     throughout: the two heads' score matmuls (hs=64 contractions) run
     CONCURRENTLY in the upper/lower half of the PE array, and the PV
     matmuls are split into two concurrent 64-kpos halves, so the score
     step costs half its untiled time and no mode switch occurs inside
     the loop.  Per-k-block causal trim on exp.
  4. One AllToAll ships both heads' unnormalized output+l; the fc
     weights prefetch into SBUF during the collective's dead window.
  5. proj (+1/l), LN2 (stats interleaved into the proj loop), MLP.

All matmul operands are bf16 (f32r moving streams at half rate).
"""
import numpy as np
import ml_dtypes

import concourse.bass as bass
import concourse.bacc as bacc
import concourse.tile as tile
import concourse.mybir as mybir
from concourse import bass_utils

F32 = mybir.dt.float32
F32R = mybir.dt.float32r
BF16 = mybir.dt.bfloat16
AF = mybir.ActivationFunctionType
OP = mybir.AluOpType

NCORES = 8
C = 1024
T = 4096
TC = T // NCORES          # 512 own T columns
CK = C // 128             # 8 C chunks
HS = 64
FC = 4096                 # MLP hidden
EPS = 1e-5

_CACHE = {}
DEBUG = False


def _build():
    nc = bacc.Bacc("TRN2", target_bir_lowering=False, debug=False,
                   enable_asserts=False, num_devices=NCORES)

    xT = nc.dram_tensor("xT", [C, TC], F32, kind="ExternalInput").ap()
    xb = nc.dram_tensor("xb", [128, CK, T], BF16, kind="ExternalInput").ap()
    wqkv = nc.dram_tensor("wqkv", [128, CK, 3 * 128], BF16,
                          kind="ExternalInput").ap()
    wqsn = nc.dram_tensor("wqsn", [1, 3 * 128], BF16, kind="ExternalInput").ap()
    wproj = nc.dram_tensor("wproj", [128, CK, C], BF16,
                           kind="ExternalInput").ap()
    wfc = nc.dram_tensor("wfc", [128, FC // 128, CK, 128], BF16,
                         kind="ExternalInput").ap()
    wmlp = nc.dram_tensor("wmlp", [128, FC // 128, C], BF16,
                          kind="ExternalInput").ap()
    ln2w = nc.dram_tensor("ln2w", [128, CK], F32, kind="ExternalInput").ap()
    masks = nc.dram_tensor("masks", [128, 4 * 512], BF16,
                           kind="ExternalInput").ap()
    ident = nc.dram_tensor("ident", [128, 128], BF16, kind="ExternalInput").ap()
    onesw = nc.dram_tensor("onesw", [128, 128], F32, kind="ExternalInput").ap()
    sel8a = nc.dram_tensor("sel8a", [16, CK * 128], BF16,
                           kind="ExternalInput").ap()
    sel8b = nc.dram_tensor("sel8b", [16, CK * 128], BF16,
                           kind="ExternalInput").ap()
    outT = nc.dram_tensor("outT", [C, TC], F32, kind="ExternalOutput").ap()
    dbg = {}
    if DEBUG:
        for nm in ("d_qp0", "d_qp1", "d_kT", "d_vT"):
            dbg[nm] = nc.dram_tensor(nm, [128, T], BF16,
                                     kind="ExternalOutput").ap()
        dbg["d_oTe0"] = nc.dram_tensor("d_oTe0", [65, T], BF16,
                                       kind="ExternalOutput").ap()
        dbg["d_oTe1"] = nc.dram_tensor("d_oTe1", [65, T], BF16,
                                       kind="ExternalOutput").ap()
        dbg["d_x2T"] = nc.dram_tensor("d_x2T", [128, CK * 512], F32,
                                      kind="ExternalOutput").ap()

    rg = [list(range(NCORES))]

    with tile.TileContext(nc) as tc:
        with tc.tile_pool(name="dram", bufs=1, space="DRAM") as dramp:
            mrb = dramp.tile([1, 1024], BF16, name="mrb")
            mrall_d = dramp.tile([NCORES, 1024], BF16, name="mrall_d",
                                 addr_space="Shared")
            # AllToAll payload: per 130-row block j (-> core j): head-a 64
            # o rows + l row, then head-b 64 o rows + l row, columns blk j.
            ob = dramp.tile([NCORES * 130, TC], BF16, name="ob")
            oax = dramp.tile([NCORES * 130, TC], BF16, name="oax")

            with tc.tile_pool(name="glob", bufs=1) as gp:
                ident_t = gp.tile([128, 128], BF16, name="ident_t")
                nc.sync.dma_start(ident_t[:], ident[:])
                ones_t = gp.tile([128, 128], F32R, name="ones_t")
                nc.sync.dma_start(ones_t[:], onesw[:].bitcast(F32R))
                ones_b = gp.tile([128, 128], BF16, name="ones_b")
                nc.vector.memset(ones_b[:], 1.0)
                sel8a_t = gp.tile([16, CK * 128], BF16, name="sel8a_t")
                nc.sync.dma_start(sel8a_t[:], sel8a[:])
                sel8b_t = gp.tile([16, CK * 128], BF16, name="sel8b_t")
                nc.sync.dma_start(sel8b_t[:], sel8b[:])
                ln2w_t = gp.tile([128, CK], F32, name="ln2w_t")
                nc.sync.dma_start(ln2w_t[:], ln2w[:])
                masks_t = gp.tile([128, 4 * 512], BF16, name="masks_t")
                nc.sync.dma_start(masks_t[:], masks[:])
                eps_t = gp.tile([128, 1], F32, name="eps_t")
                nc.vector.memset(eps_t[:], EPS)
                wq_t = gp.tile([128, CK, 3 * 128], BF16, name="wq_t")
                nc.sync.dma_start(wq_t[:], wqkv[:])
                wqsn_t = gp.tile([1, 3 * 128], BF16, name="wqsn_t")
                nc.sync.dma_start(wqsn_t[:], wqsn[:])
                xT_t = gp.tile([128, CK, 512], F32R, name="xT_t")
                nc.sync.dma_start(
                    xT_t[:], xT.rearrange("(k p) c -> p k c", p=128)
                    .bitcast(F32R))
                x2T_t = gp.tile([128, CK, 512], F32R, name="x2T_t")

                # ------------- attention-data scope -------------
                adp = tc.alloc_tile_pool(name="adp", bufs=1)
                qp0 = adp.tile([128, T], BF16, name="qp0")
                qp1 = adp.tile([128, T], BF16, name="qp1")
                kT_t = adp.tile([128, T], BF16, name="kT_t")
                vT_t = adp.tile([128, T], BF16, name="vT_t")
                oTe0 = adp.tile([65, T], BF16, name="oTe0")
                oTe1 = adp.tile([65, T], BF16, name="oTe1")
                mr = adp.tile([1, 1024], BF16, name="mr")
                mr_js = [adp.tile([1, 1024], BF16, name=f"mr{j}")
                         for j in range(NCORES)]
                ve = [[adp.tile([128, 65], BF16, name=f"ve{h}_{t}")
                       for t in range(T // 128)] for h in range(2)]

                # LN1 stats for the own block -> mr [1,1024] = (rstd, mu)
                with tc.tile_pool(name="ln1", bufs=1) as lnp, \
                     tc.tile_pool(name="lnps1", bufs=1, space="PSUM") as lps:
                    mean_ps = lps.tile([128, 512], F32, name="mean1")
                    sq_ps = lps.tile([128, 512], F32, name="sqs1")
                    sq_tiles = []
                    for k in range(CK):
                        sq = lnp.tile([128, 512], BF16, name="sq1", bufs=2)
                        xk = xT_t[:, k, :].bitcast(F32)
                        nc.vector.tensor_mul(sq[:], xk, xk)
                        sq_tiles.append(sq)
                    for k in range(CK):
                        nc.tensor.matmul(mean_ps[:], ones_t[:],
                                         xT_t[:, k, :],
                                         start=(k == 0), stop=(k == CK - 1))
                    for k in range(CK):
                        nc.tensor.matmul(sq_ps[:], ones_b[:],
                                         sq_tiles[k][:],
                                         start=(k == 0), stop=(k == CK - 1))
                    mu = lnp.tile([1, 512], F32, name="mu1")
                    nc.vector.tensor_scalar_mul(mu[:], mean_ps[0:1, :],
                                                1.0 / C)
                    musq = lnp.tile([1, 512], F32, name="musq1")
                    nc.vector.tensor_mul(musq[:], mu[:], mu[:])
                    var = lnp.tile([1, 512], F32, name="var1")
                    nc.vector.scalar_tensor_tensor(
                        var[:], sq_ps[0:1, :], 1.0 / C, musq[:],
                        OP.mult, OP.subtract)
                    lnv = lnp.tile([1, 512], F32, name="lnv1")
                    nc.scalar.activation(lnv[:], var[:], AF.Ln,
                                         bias=eps_t[0:1, :])
                    nc.scalar.activation(mr[:, 0:512], lnv[:], AF.Exp,
                                         scale=-0.5)
                    nc.vector.tensor_copy(mr[:, 512:1024], mu[:])
                nc.sync.dma_start(mrb[:], mr[:])
                nc.gpsimd.collective_compute(
                    "AllGather", OP.bypass, replica_groups=rg,
                    ins=[mrb.opt()], outs=[mrall_d.opt()])

                # ---- QKV raw accumulation (independent of AllGather):
                # unscaled sum_k W'[:,k,m]^T x[k, blk], evacuated straight
                # into the final q/k/v tiles; LN scale applied in place
                # once the AllGather'd (rstd, mu) land.
                with tc.tile_pool(name="xbp", bufs=1) as xbp, \
                     tc.tile_pool(name="qkvps", bufs=1, space="PSUM") as qps:
                    xb_t = xbp.tile([128, CK, T], BF16, name="xb_t")
                    for j in range(NCORES):
                        nc.sync.dma_start(
                            xb_t[:, :, 512 * j:512 * (j + 1)],
                            xb[:, :, 512 * j:512 * (j + 1)])
                    for j in range(NCORES):
                        blk = slice(512 * j, 512 * (j + 1))
                        for m in range(3):
                            msl = slice(128 * m, 128 * (m + 1))
                            pm = qps.tile([128, 512], F32, name="qkvp",
                                          bufs=3)
                            for k in range(CK):
                                nc.tensor.matmul(
                                    pm[:], wq_t[:, k, msl],
                                    xb_t[:, k, blk],
                                    start=(k == 0), stop=(k == CK - 1))
                            if m == 0:
                                nc.scalar.activation(
                                    qp0[0:64, blk], pm[0:64, :], AF.Copy)
                                nc.scalar.activation(
                                    qp1[64:128, blk], pm[64:128, :],
                                    AF.Copy)
                            elif m == 1:
                                nc.scalar.activation(kT_t[:, blk], pm[:],
                                                     AF.Copy)
                            else:
                                nc.scalar.activation(vT_t[:, blk], pm[:],
                                                     AF.Copy)

                for j in range(NCORES):
                    nc.sync.dma_start(mr_js[j][:], mrall_d[j:j + 1, :])

                halves = [(qp0, slice(0, 64)), (qp1, slice(64, 128))]
                qdst = [None, kT_t, vT_t]

                # ---- LN finalize (in place) + v transposes.  GPSIMD does
                # the scale multiplies; v is finalized first across all
                # blocks so the transposes (PE) unblock early.
                with tc.tile_pool(name="finp", bufs=1) as fnp, \
                     tc.tile_pool(name="finps", bufs=1, space="PSUM") as fps:
                    rb_t = []
                    for j in range(NCORES):
                        bps = fps.tile([128, 512], F32, name="fpb", bufs=1)
                        nc.tensor.matmul(bps[:], ones_b[0:1, :],
                                         mr_js[j][:, 0:512],
                                         start=True, stop=True)
                        rb = fnp.tile([128, 512], BF16, name="rb", bufs=8)
                        nc.scalar.activation(rb[:], bps[:], AF.Copy)
                        rb_t.append(rb)

                    def fin_jm(j, m):
                        blk = slice(512 * j, 512 * (j + 1))
                        msl = slice(128 * m, 128 * (m + 1))
                        pmb = fps.tile([128, 512], F32, name="fpb", bufs=1)
                        nc.tensor.matmul(pmb[:], wqsn_t[:, msl],
                                         mr_js[j][:, 512:1024],
                                         start=True, stop=True)
                        tt = fnp.tile([128, 512], BF16, name="tt", bufs=4)
                        if m == 0:
                            for dst, hs in halves:
                                nc.vector.tensor_add(
                                    tt[hs, :], pmb[hs, :], dst[hs, blk])
                                nc.gpsimd.tensor_mul(
                                    dst[hs, blk], tt[hs, :], rb_t[j][hs, :])
                        else:
                            nc.vector.tensor_add(tt[:], pmb[:],
                                                 qdst[m][:, blk])
                            nc.gpsimd.tensor_mul(
                                qdst[m][:, blk], tt[:], rb_t[j][:])

                    # v first (unblocks transposes), then k, then q.
                    for j in range(NCORES):
                        fin_jm(j, 2)
                    for j in range(NCORES):
                        fin_jm(j, 1)
                        fin_jm(j, 0)
                    # v_ext transposes: [kpos, 64 dims]+ones per head
                    for t in range(T // 128):
                        tp = fps.tile([128, 128], BF16, name="vtp", bufs=2)
                        nc.tensor.transpose(
                            tp[:], vT_t[:, 128 * t:128 * (t + 1)], ident_t[:])
                        for h in range(2):
                            vx = ve[h][t]
                            nc.vector.tensor_copy(
                                vx[:, 0:64], tp[:, 64 * h:64 * (h + 1)])
                            nc.vector.memset(vx[:, 64:65], 1.0)

                # ---- flash attention (no max subtraction), both heads per
                # q-block; the PE stays in 64-row tiling mode: score mms for
                # the two heads run concurrently (upper/lower array half),
                # PV is split into two concurrent 64-kpos halves.
                with tc.tile_pool(name="atw", bufs=1) as aw, \
                     tc.tile_pool(name="atps", bufs=1, space="PSUM") as aps, \
                     tc.tile_pool(name="atpo", bufs=1, space="PSUM") as apo:
                    for qi in range(NCORES):
                        nkb = 4 * (qi + 1)
                        qsl = slice(512 * qi, 512 * (qi + 1))
                        oph = [[apo.tile([65, 512], F32, name=f"op{h}{u}",
                                         bufs=1) for u in range(2)]
                               for h in range(2)]
                        for kb in range(nkb):
                            s = kb - 4 * qi
                            q0 = 128 * s if s >= 0 else 0
                            ksl = slice(128 * kb, 128 * (kb + 1))
                            sp = aps.tile([128, 2, 512], F32, name="sp",
                                          bufs=2)
                            nc.tensor.matmul(
                                sp[:, 0, q0:512], kT_t[0:64, ksl],
                                qp0[0:64, 512 * qi + q0:512 * (qi + 1)],
                                start=True, stop=True)
                            nc.tensor.matmul(
                                sp[:, 1, q0:512], kT_t[64:128, ksl],
                                qp1[64:128, 512 * qi + q0:512 * (qi + 1)],
                                start=True, stop=True)
                            est = aw.tile([128, 2, 512], BF16, name="est",
                                          bufs=4)
                            if q0:
                                nc.vector.memset(est[:, :, 0:q0], 0.0)
                            nc.scalar.activation(est[:, :, q0:512],
                                                 sp[:, :, q0:512],
                                                 AF.Exp, scale=0.125)
                            if s >= 0:  # diagonal: causal mask
                                for u in range(2):
                                    nc.vector.tensor_mul(
                                        est[:, u, q0:512],
                                        est[:, u, q0:512],
                                        masks_t[:, 512 * s + q0:
                                                512 * (s + 1)])
                            for h in range(2):
                                for u, psl in ((0, slice(0, 64)),
                                               (1, slice(64, 128))):
                                    nc.tensor.matmul(
                                        oph[h][u][:], ve[h][kb][psl, :],
                                        est[psl, h, :],
                                        start=(kb == 0),
                                        stop=(kb == nkb - 1))
                        for h, oTeh in ((0, oTe0), (1, oTe1)):
                            lo = aw.tile([65, 512], BF16, name=f"lo{h}",
                                         bufs=2)
                            nc.vector.tensor_copy(lo[:], oph[h][1][:])
                            nc.vector.tensor_add(oTeh[:, qsl],
                                                 oph[h][0][:], lo[:])

                # ship both heads' output+l in one AllToAll
                ob_r = ob.rearrange("(j p) c -> p j c", p=130)
                nc.sync.dma_start(
                    ob_r[0:65], oTe0[:].rearrange("p (j c) -> p j c",
                                                  j=NCORES))
                nc.sync.dma_start(
                    ob_r[65:130], oTe1[:].rearrange("p (j c) -> p j c",
                                                    j=NCORES))
                if DEBUG:
                    nc.sync.dma_start(dbg["d_qp0"][:], qp0[:])
                    nc.sync.dma_start(dbg["d_qp1"][:], qp1[:])
                    nc.sync.dma_start(dbg["d_kT"][:], kT_t[:])
                    nc.sync.dma_start(dbg["d_vT"][:], vT_t[:])
                    nc.sync.dma_start(dbg["d_oTe0"][:], oTe0[:])
                    nc.sync.dma_start(dbg["d_oTe1"][:], oTe1[:])
                nc.gpsimd.collective_compute(
                    "AllToAll", OP.bypass, replica_groups=rg,
                    ins=[ob.opt()], outs=[oax.opt()])
                adp.release()

                # fc weights prefetch: the DMA fills the SBUF freed by adp
                # while the AllToAll is in flight.
                wfcp = tc.alloc_tile_pool(name="wfcp", bufs=1)
                wfc_t = wfcp.tile([128, FC // 128, CK, 128], BF16,
                                  name="wfc_t")
                h2T_t = wfcp.tile([128, CK, 512], BF16, name="h2T_t")

                # proj: x2T = xT + wproj.T @ (aoutT * 1/l), with the LN2
                # statistics matmuls interleaved per output chunk.
                wpp = tc.alloc_tile_pool(name="wpp", bufs=1)
                wp_t = wpp.tile([128, CK, C], BF16, name="wp_t")
                nc.sync.dma_start(wp_t[:], wproj[:])
                for g in range(FC // 512):
                    nc.sync.dma_start(wfc_t[:, 4 * g:4 * g + 4],
                                      wfc[:, 4 * g:4 * g + 4])
                with tc.tile_pool(name="prs", bufs=1) as prs, \
                     tc.tile_pool(name="prps", bufs=1, space="PSUM") as pps, \
                     tc.tile_pool(name="lnps2", bufs=1, space="PSUM") as lps2:
                    at_tiles = [prs.tile([128, 512], BF16, name=f"at{k}")
                                for k in range(CK)]
                    oax_r = oax.rearrange("(k p) c -> p k c", p=130)
                    lax_r = oax.rearrange("(k p) c -> k p c", p=130)
                    for hx, sel_h in enumerate([sel8a_t, sel8b_t]):
                        hsl = slice(64 * hx, 64 * (hx + 1))
                        au = prs.tile([64, CK, 512], BF16, name=f"au{hx}")
                        nc.sync.dma_start(
                            au[:], oax_r[65 * hx:65 * hx + 64])
                        lh = prs.tile([8, 512], BF16, name=f"l{hx}")
                        nc.sync.dma_start(lh[:], lax_r[:, 65 * hx + 64, :])
                        lln = prs.tile([8, 512], F32, name=f"lln{hx}")
                        nc.scalar.activation(lln[:], lh[:], AF.Ln)
                        rl = prs.tile([8, 512], BF16, name=f"rl{hx}")
                        nc.scalar.activation(rl[:], lln[:], AF.Exp,
                                             scale=-1.0)
                        for k in range(CK):
                            rlb = pps.tile([128, 512], F32, name="rlb",
                                           bufs=2)
                            nc.tensor.matmul(
                                rlb[:], sel_h[0:8, 128 * k:128 * (k + 1)],
                                rl[:], start=True, stop=True)
                            nc.vector.tensor_mul(
                                at_tiles[k][hsl, :], au[:, k, :],
                                rlb[hsl, :])
                    # proj + interleaved LN2 stats
                    mean2 = lps2.tile([128, 512], F32, name="mean2")
                    sqs2 = lps2.tile([128, 512], F32, name="sqs2")
                    sq2_tiles = []
                    for m in range(CK):
                        x2ps = pps.tile([128, 512], F32, name="x2p", bufs=2)
                        for k in range(CK):
                            nc.tensor.matmul(
                                x2ps[:], wp_t[:, k, 128 * m:128 * (m + 1)],
                                at_tiles[k][:], start=(k == 0),
                                stop=(k == CK - 1))
                        nc.vector.tensor_add(
                            x2T_t[:, m, :], x2ps[:],
                            xT_t[:, m, :].bitcast(F32))
                        sq2 = prs.tile([128, 512], BF16, name="sq2", bufs=8)
                        x2m = x2T_t[:, m, :].bitcast(F32)
                        nc.vector.tensor_mul(sq2[:], x2m, x2m)
                        sq2_tiles.append(sq2)
                        nc.tensor.matmul(mean2[:], ones_t[:],
                                         x2T_t[:, m, :],
                                         start=(m == 0), stop=(m == CK - 1))
                    for m in range(CK):
                        nc.tensor.matmul(sqs2[:], ones_b[:],
                                         sq2_tiles[m][:],
                                         start=(m == 0), stop=(m == CK - 1))
                    # LN2 scalars + normalized h2 (bf16)
                    mu2 = prs.tile([128, 512], F32, name="mu2")
                    nc.vector.tensor_scalar_mul(mu2[:], mean2[:], 1.0 / C)
                    musq2 = prs.tile([128, 512], F32, name="musq2")
                    nc.vector.tensor_mul(musq2[:], mu2[:], mu2[:])
                    var2 = prs.tile([128, 512], F32, name="var2")
                    nc.vector.scalar_tensor_tensor(
                        var2[:], sqs2[:], 1.0 / C, musq2[:],
                        OP.mult, OP.subtract)
                    lnv2 = prs.tile([128, 512], F32, name="lnv2")
                    nc.scalar.activation(lnv2[:], var2[:], AF.Ln,
                                         bias=eps_t[:])
                    rstd2 = prs.tile([128, 512], F32, name="rstd2")
                    nc.scalar.activation(rstd2[:], lnv2[:], AF.Exp,
                                         scale=-0.5)
                    for k in range(CK):
                        d2 = prs.tile([128, 512], F32, name="d2", bufs=2)
                        nc.vector.tensor_sub(d2[:],
                                             x2T_t[:, k, :].bitcast(F32),
                                             mu2[:])
                        nc.vector.scalar_tensor_tensor(
                            h2T_t[:, k, :], d2[:], ln2w_t[:, k:k + 1],
                            rstd2[:], OP.mult, OP.mult)
                if DEBUG:
                    nc.sync.dma_start(
                        dbg["d_x2T"][:],
                        x2T_t[:].bitcast(F32).rearrange("p k c -> p (k c)"))
                wpp.release()

                # ---------------- MLP ----------------
                with tc.tile_pool(name="mlp", bufs=1) as mp:
                    gel = []
                    with tc.tile_pool(name="fcps", bufs=1,
                                      space="PSUM") as fps2:
                        for g in range(FC // 512):  # 8 groups of 4 m-blocks
                            pf = fps2.tile([128, 4, 512], F32, name="fcp",
                                           bufs=2)
                            for mm in range(4):
                                for k in range(CK):
                                    nc.tensor.matmul(
                                        pf[:, mm, :],
                                        wfc_t[:, 4 * g + mm, k, :],
                                        h2T_t[:, k, :],
                                        start=(k == 0), stop=(k == CK - 1))
                            gl = mp.tile([128, 4, 512], BF16, name=f"gel{g}")
                            nc.scalar.activation(gl[:], pf[:], AF.Gelu)
                            gel.append(gl)
                    # second matmul: single pass, 8 psum accumulators
                    with tc.tile_pool(name="m2s", bufs=1) as m2s, \
                         tc.tile_pool(name="m2ps", bufs=1,
                                      space="PSUM") as m2ps:
                        x3ps = [m2ps.tile([128, 512], F32, name=f"x3p{i}")
                                for i in range(CK)]
                        for f4 in range(FC // 512):
                            wm = m2s.tile([128, 4, C], BF16, name="wm",
                                          bufs=2)
                            nc.sync.dma_start(
                                wm[:], wmlp[:, 4 * f4:4 * f4 + 4, :])
                            for ff in range(4):
                                f = 4 * f4 + ff
                                for i in range(CK):
                                    nc.tensor.matmul(
                                        x3ps[i][:],
                                        wm[:, ff, 128 * i:128 * (i + 1)],
                                        gel[f // 4][:, f % 4, :],
                                        start=(f == 0),
                                        stop=(f == FC // 128 - 1))
                        for i in range(CK):
                            o32 = m2s.tile([128, 512], F32, name="o32",
                                           bufs=2)
                            nc.vector.tensor_add(
                                o32[:], x3ps[i][:],
                                x2T_t[:, i, :].bitcast(F32))
                            nc.sync.dma_start(
                                outT[128 * i:128 * (i + 1), :], o32[:])
                wfcp.release()

    nc.compile()
    return nc


def _host_inputs(x, w_qkv, w_attn_proj, w_fc, w_mlp_proj, ln1_w, ln2_w):
    """Build the 8 per-core input maps."""
    bf = ml_dtypes.bfloat16
    x2 = np.ascontiguousarray(np.asarray(x, np.float32).reshape(T, C))
    w_qkv = np.asarray(w_qkv, np.float32)
    ln1_w = np.asarray(ln1_w, np.float32)
    masks = np.zeros((128, 4 * 512), np.float32)
    kk = np.arange(128)[:, None]
    qq = np.arange(512)[None, :]
    for j in range(4):
        masks[:, 512 * j:512 * (j + 1)] = (qq >= kk + 128 * j)
    masks = masks.astype(bf)
    ident = np.eye(128, dtype=np.float32).astype(bf)
    onesw = np.ones((128, 128), np.float32)
    # sel8a[k, 128k+d] = 1 for d<64; sel8b[k, 128k+d] = 1 for d>=64.
    # Broadcasts per-head 1/l rows onto that head's 64 dim-rows.
    # (16 rows: input DMAs below 16 partitions corrupt other tiles.)
    sel8a = np.zeros((16, CK * 128), np.float32)
    sel8b = np.zeros((16, CK * 128), np.float32)
    for k in range(CK):
        sel8a[k, 128 * k:128 * k + 64] = 1.0
        sel8b[k, 128 * k + 64:128 * (k + 1)] = 1.0
    sel8a = sel8a.astype(bf)
    sel8b = sel8b.astype(bf)
    ln2 = np.ascontiguousarray(np.asarray(ln2_w, np.float32).reshape(CK, 128).T)
    # full x, transposed + C-chunked, bf16: xb[p, k, t] = x[t, 128k+p]
    xball = np.ascontiguousarray(
        x2.T.reshape(CK, 128, T).transpose(1, 0, 2).astype(bf))
    wproj = np.asarray(w_attn_proj, np.float32).reshape(CK, 128, C) \
        .transpose(1, 0, 2).astype(bf)
    wfc = np.asarray(w_fc, np.float32).reshape(CK, 128, FC // 128, 128) \
        .transpose(1, 2, 0, 3).astype(bf)
    wmlp = np.asarray(w_mlp_proj, np.float32).reshape(FC // 128, 128, C) \
        .transpose(1, 0, 2).astype(bf)
    common = {
        "xb": xball,
        "wproj": np.ascontiguousarray(wproj),
        "wfc": np.ascontiguousarray(wfc),
        "wmlp": np.ascontiguousarray(wmlp),
        "ln2w": ln2, "masks": masks, "ident": ident,
        "onesw": onesw, "sel8a": sel8a, "sel8b": sel8b,
    }
    in_maps = []
    for c in range(NCORES):
        xTc = np.ascontiguousarray(x2[TC * c:TC * (c + 1), :].T)
        wq = np.concatenate(
            [w_qkv[:, C * s + 128 * c:C * s + 128 * (c + 1)] for s in range(3)],
            axis=1)  # [C, 384] pre-folded with ln1 weight
        wq = wq * ln1_w[:, None]
        wqs = wq.sum(axis=0)  # [384]
        wqsn = np.ascontiguousarray((-wqs).reshape(1, 3 * 128).astype(bf))
        wq = np.ascontiguousarray(
            wq.reshape(CK, 128, 3 * 128).transpose(1, 0, 2).astype(bf))
        in_maps.append({"xT": xTc, "wqkv": wq, "wqsn": wqsn, **common})
    return in_maps


def _run(in_maps, **kw):
    key = ("nc", DEBUG)
    if key not in _CACHE:
        _CACHE[key] = _build()
    return bass_utils.run_bass_kernel_spmd(
        _CACHE[key], in_maps, core_ids=list(range(NCORES)), **kw)


def kernel(x, w_qkv, w_attn_proj, w_fc, w_mlp_proj, ln1_w, ln2_w):
    in_maps = _host_inputs(x, w_qkv, w_attn_proj, w_fc, w_mlp_proj,
                           ln1_w, ln2_w)
    res = _run(in_maps)
    out = np.empty((1, T, C), np.float32)
    for c in range(NCORES):
        out[0, TC * c:TC * (c + 1), :] = res.results[c]["outT"].T
    return out
